# revision 48
# baseline (speedup 1.0000x reference)
"""Self-contained Trainium2 Bass kernel for nn_Attention_62560493633940.

Sharding: 16 heads split across 8 cores (2 q-heads + their shared kv-head
per core, tensor parallel); x / pos replicated; per-core partial output
projections (over that core's 128 o-columns) summed on host.

Math notes:
 - pos_logits[h,q,k] = a[q,h] - a[k,h] + bh[h] with a = p @ Wh.T, so
   softmax_k(pos_logits) is independent of q (shift invariance) ->
   pos_attn is a rank-1 per-head key distribution.  Constant-in-k shifts
   cancel in that softmax, so bh AND the bp2-induced shift drop out.
 - Both softmaxes sum to 1, so the re-normalization is an identity and
   the mix is (1-g)*attn + g*pos_attn.
 - fp8 (e4m3) DoubleRow matmuls (2 rows/cycle) are used for the q/k/v
   projections, the QK^T logits and the E@V accumulation.  v keeps
   ~f16 accuracy via a hi+lo fp8 split of both x and Wv (3 chains);
   the dropped (xr*Wvr) term is ~0.1% relative.
 - The "ones" column of the fp8 V-augmented matrix is 4.0 (exact in
   e4m3); host constants fold the 1/4 and the per-head gate factors.
"""
import sys

if '/opt/trn_rl_repo' not in sys.path:
    sys.path.insert(0, '/opt/trn_rl_repo')

import numpy as np
import ml_dtypes

import concourse.bass as bass
import concourse.bacc as bacc
import concourse.tile as tile
import concourse.mybir as mybir
from concourse import bass_utils
from concourse.masks import make_identity

F32 = mybir.dt.float32
F16 = mybir.dt.float16
F8 = mybir.dt.float8e4
F8E5 = mybir.dt.float8e5
E4NP = ml_dtypes.float8_e4m3
E5NP = ml_dtypes.float8_e5m2

T = 1024      # sequence length
DIM = 1024    # model dim
H = 16        # heads
KVH = 4       # kv heads
HD = 64       # head dim
PD = 64       # pos dim
PF = 128      # pos feature dim
BASE = 10000.0
NC = 8        # cores

DR = mybir.MatmulPerfMode.DoubleRow

_CACHE = {}


def _build_program(reps=1, stage=99):
    nc = bacc.Bacc("TRN2")

    # ---- DRAM parameters (per-core data arrives via in_maps) ----
    x8_d = nc.declare_dram_parameter("x8", [128, 8 * T], F8, isOutput=False)
    xr8_d = nc.declare_dram_parameter("xr8", [128, 8 * T], F8, isOutput=False)
    wq8_d = nc.declare_dram_parameter("wq8", [128, DIM], F8, isOutput=False)
    wkvA_d = nc.declare_dram_parameter("wkvA", [128, DIM], F8, isOutput=False)
    wkvB_d = nc.declare_dram_parameter("wkvB", [128, DIM], F8E5, isOutput=False)
    wo_d = nc.declare_dram_parameter("wo", [128, DIM], F16, isOutput=False)
    posTa_d = nc.declare_dram_parameter("posTa", [PD + 1, T], F16, isOutput=False)
    wp1Ta_d = nc.declare_dram_parameter("wp1Ta", [PD + 1, PD], F16, isOutput=False)
    wp2T_d = nc.declare_dram_parameter("wp2T", [PD, PF], F16, isOutput=False)
    whT2_d = nc.declare_dram_parameter("whT2", [PF, 32], F16, isOutput=False)
    c64_d = nc.declare_dram_parameter("c64", [64, 2], F32, isOutput=False)
    sg2m_d = nc.declare_dram_parameter("sg2m", [2, 1], F32, isOutput=False)
    ones8_d = nc.declare_dram_parameter("ones8", [128, 128], F8, isOutput=False)
    ones16_d = nc.declare_dram_parameter("ones16", [128, 4], F16, isOutput=False)
    tabc_d = nc.declare_dram_parameter("tabc", [128, T], F16, isOutput=False)
    tabs_d = nc.declare_dram_parameter("tabs", [128, T], F16, isOutput=False)
    outp_d = nc.declare_dram_parameter("outp", [T, DIM], F16, isOutput=True)

    ExpF = mybir.ActivationFunctionType.Exp
    AluMult = mybir.AluOpType.mult
    AluAdd = mybir.AluOpType.add
    AluMax = mybir.AluOpType.max

    with tile.TileContext(nc) as tc:
        with tc.tile_pool(name="cst", bufs=1) as cst, \
             tc.tile_pool(name="wk", bufs=1) as wk, \
             tc.tile_pool(name="eP", bufs=2) as eP, \
             tc.tile_pool(name="oP", bufs=2) as oP, \
             tc.tile_pool(name="vP", bufs=4) as vP, \
             tc.tile_pool(name="psW", bufs=2, space="PSUM") as psW, \
             tc.tile_pool(name="psA", bufs=2, space="PSUM") as psA:

            for _rep in range(reps):
                # ---- constants into SBUF ----
                x8_sb = cst.tile([128, 8, T], F8)
                x8_ap = x8_d.ap().rearrange("p (k t) -> p k t", k=8)
                for kk in range(0, 8, 4):
                    nc.gpsimd.dma_start(out=x8_sb[:, kk:kk + 4, :],
                                        in_=x8_ap[:, kk:kk + 4, :])
                xr8_sb = cst.tile([128, 8, T], F8)
                xr8_ap = xr8_d.ap().rearrange("p (k t) -> p k t", k=8)
                for kk in range(0, 8, 4):
                    nc.gpsimd.dma_start(out=xr8_sb[:, kk:kk + 4, :],
                                        in_=xr8_ap[:, kk:kk + 4, :])
                wq8_sb = cst.tile([128, 8, 128], F8)
                nc.sync.dma_start(out=wq8_sb[:],
                                  in_=wq8_d.ap().rearrange("p (k m) -> p k m", k=8))
                wkvA_sb = cst.tile([128, 8, 128], F8)
                nc.sync.dma_start(out=wkvA_sb[:],
                                  in_=wkvA_d.ap().rearrange("p (k m) -> p k m", k=8))
                wkvB_sb = cst.tile([128, 8, 128], F8E5)
                nc.sync.dma_start(out=wkvB_sb[:],
                                  in_=wkvB_d.ap().rearrange("p (k m) -> p k m", k=8))
                posTa_sb = cst.tile([PD + 1, T], F16)
                nc.gpsimd.dma_start(out=posTa_sb[:], in_=posTa_d.ap())
                wp1Ta_sb = cst.tile([PD + 1, PD], F16)
                nc.gpsimd.dma_start(out=wp1Ta_sb[:], in_=wp1Ta_d.ap())
                wp2T_sb = cst.tile([PD, PF], F16)
                nc.gpsimd.dma_start(out=wp2T_sb[:], in_=wp2T_d.ap())
                whT2_sb = cst.tile([PF, 32], F16)
                nc.gpsimd.dma_start(out=whT2_sb[:], in_=whT2_d.ap())
                c64_sb = cst.tile([64, 2], F32)
                nc.sync.dma_start(out=c64_sb[:], in_=c64_d.ap())
                sg2m_sb = cst.tile([2, 1], F32)
                nc.sync.dma_start(out=sg2m_sb[:], in_=sg2m_d.ap())
                ones8_sb = cst.tile([128, 128], F8)
                nc.sync.dma_start(out=ones8_sb[:], in_=ones8_d.ap())
                ones16_sb = cst.tile([128, 4], F16)
                nc.sync.dma_start(out=ones16_sb[:], in_=ones16_d.ap())
                tabc_sb = cst.tile([128, T], F16)
                nc.sync.dma_start(out=tabc_sb[:], in_=tabc_d.ap())
                tabs_sb = cst.tile([128, T], F16)
                nc.sync.dma_start(out=tabs_sb[:], in_=tabs_d.ap())

                ones64r = cst.tile([1, 64], F16)
                nc.vector.memset(ones64r, 1.0)
                id64f = cst.tile([64, 64], F32)
                make_identity(nc, id64f)
                id64r = cst.tile([64, 64], F16)
                nc.vector.tensor_scalar_mul(id64r[:], id64f[:], 1.0)

                wo_sb = cst.tile([128, DIM], F16)
                nc.sync.dma_start(out=wo_sb[:], in_=wo_d.ap())

                # ---- q / kv projections (fp8 DoubleRow; rows = proj dims) --
                qraw = psW.tile([128, T], F32, tag="wide")
                kvraw = psW.tile([128, T], F32, tag="wide")
                for n in range(2):
                    sl = slice(512 * n, 512 * n + 512)
                    for p in range(4):
                        nc.tensor.matmul(qraw[:, sl],
                                         wq8_sb[:, 2 * p:2 * p + 2, :],
                                         x8_sb[:, 2 * p:2 * p + 2, sl],
                                         start=(p == 0), stop=(p == 3),
                                         perf_mode=DR)
                    for p in range(4):
                        nc.tensor.matmul(kvraw[:, sl],
                                         wkvA_sb[:, 2 * p:2 * p + 2, :],
                                         x8_sb[:, 2 * p:2 * p + 2, sl],
                                         start=(p == 0), stop=False,
                                         perf_mode=DR)
                    for p in range(4):
                        nc.tensor.matmul(kvraw[:, sl],
                                         wkvA_sb[:, 2 * p:2 * p + 2, :],
                                         xr8_sb[:, 2 * p:2 * p + 2, sl],
                                         start=False, stop=False,
                                         perf_mode=DR)
                    for p in range(4):
                        nc.tensor.matmul(kvraw[:, sl],
                                         wkvB_sb[:, 2 * p:2 * p + 2, :],
                                         x8_sb[:, 2 * p:2 * p + 2, sl],
                                         start=False, stop=(p == 3),
                                         perf_mode=DR)

                if stage < 1:
                    continue
                # ---- RoPE on q -> q8flat fp8 [128, T]
                # qraw rows: [x1_h0, x2_h0, x1_h1, x2_h1] in 32-blocks
                T1 = wk.tile([128, 2, 512], F16)
                T2 = wk.tile([128, 2, 512], F16)
                T2s = wk.tile([128, 2, 512], F16)
                q8flat = wk.tile([128, T], F8)
                qraw2 = qraw.rearrange("p (n t) -> p n t", n=2)
                nc.vector.tensor_mul(T1[:], qraw2[:], tabc_sb.rearrange(
                    "p (n t) -> p n t", n=2))
                nc.vector.tensor_mul(T2[:], qraw2[:], tabs_sb.rearrange(
                    "p (n t) -> p n t", n=2))
                for b in range(4):
                    sr = (b // 2) * 64 + (1 - (b % 2)) * 32
                    ds = (b // 2) * 64 + (b % 2) * 32
                    nc.scalar.dma_start(out=T2s[ds:ds + 32, :, :],
                                      in_=T2[sr:sr + 32, :, :])
                nc.vector.tensor_add(
                    q8flat.rearrange("p (n t) -> p n t", n=2), T1[:], T2s[:])

                # ---- RoPE on k (kvraw rows 0:64) -> k8flat fp8 [64, T]
                T1k = wk.tile([64, 2, 512], F16)
                T2k = wk.tile([64, 2, 512], F16)
                T2ks = wk.tile([64, 2, 512], F16)
                k8flat = wk.tile([64, T], F8)
                kraw2 = kvraw[0:64, :].rearrange("p (n t) -> p n t", n=2)
                nc.vector.tensor_mul(T1k[:], kraw2[:],
                                     tabc_sb[0:64, :].rearrange(
                                         "p (n t) -> p n t", n=2))
                nc.vector.tensor_mul(T2k[:], kraw2[:],
                                     tabs_sb[0:64, :].rearrange(
                                         "p (n t) -> p n t", n=2))
                nc.scalar.dma_start(out=T2ks[0:32, :, :], in_=T2k[32:64, :, :])
                nc.scalar.dma_start(out=T2ks[32:64, :, :], in_=T2k[0:32, :, :])
                nc.vector.tensor_add(
                    k8flat.rearrange("p (n t) -> p n t", n=2), T1k[:], T2ks[:])

                # rearrange to DoubleRow layout [32, 2, T]
                q8h = []
                for i in range(2):
                    q8t = wk.tile([32, 2, T], F8, name=f"q8t{i}")
                    nc.scalar.dma_start(out=q8t[:, 0, :],
                                      in_=q8flat[64 * i:64 * i + 32, :])
                    nc.scalar.dma_start(out=q8t[:, 1, :],
                                      in_=q8flat[64 * i + 32:64 * i + 64, :])
                    q8h.append(q8t)
                k8t = wk.tile([32, 2, T], F8)
                nc.scalar.dma_start(out=k8t[:, 0, :], in_=k8flat[0:32, :])
                nc.scalar.dma_start(out=k8t[:, 1, :], in_=k8flat[32:64, :])

                if stage < 2:
                    continue
                # ---- v: copy vT out of PSUM, transpose, build paired v_aug
                vT_sb = wk.tile([64, T], F16)
                nc.vector.tensor_scalar_mul(vT_sb[:], kvraw[64:128, :], 0.125)
                va8 = []
                va16 = []
                for p in range(4):
                    v8a = vP.tile([128, 2, 128], F8, tag="va8", name=f"v8a{p}")
                    v16a = vP.tile([128, 2, 66], F16, tag="va16",
                                   name=f"v16a{p}")
                    for jj in range(2):
                        m = 2 * p + jj
                        vtp = psA.tile([128, 64], F16, tag="sm",
                                       name=f"vtp{m}")
                        nc.tensor.transpose(vtp[:],
                                            vT_sb[:, 128 * m:128 * m + 128],
                                            id64r[:])
                        nc.vector.tensor_copy(v8a[:, jj, 0:64], vtp[:])
                        nc.vector.tensor_copy(v16a[:, jj, 0:64], vtp[:])
                    nc.scalar.dma_start(out=v8a[:, :, 64:128],
                                      in_=ones8_sb.rearrange(
                                          "p (j c) -> p j c", j=2))
                    nc.scalar.dma_start(out=v16a[:, :, 64:66],
                                      in_=ones16_sb.rearrange(
                                          "p (j c) -> p j c", j=2))
                    va8.append(v8a)
                    va16.append(v16a)

                # ---- pos path ----
                p1r = wk.tile([64, T], F16)
                for n in range(2):
                    sl = slice(512 * n, 512 * n + 512)
                    pp = psA.tile([64, 512], F32, tag="sm", name=f"pp{n}")
                    nc.tensor.matmul(pp[:], wp1Ta_sb[:], posTa_sb[:, sl],
                                     start=True, stop=True)
                    nc.vector.tensor_scalar(p1r[:, sl], pp[:], 0.0, None,
                                            op0=AluMax)
                p2Tb = wk.tile([PF, T], F16)
                for n in range(2):
                    sl = slice(512 * n, 512 * n + 512)
                    p2p = psA.tile([PF, 512], F32, tag="sm", name=f"p2p{n}")
                    nc.tensor.matmul(p2p[:], wp2T_sb[:], p1r[:, sl],
                                     start=True, stop=True)
                    nc.vector.tensor_copy(p2Tb[:, sl], p2p[:])
                eposAll = wk.tile([128, 256], F16)
                aALL = psA.tile([128, 256], F32, tag="sm")
                for j in range(8):
                    nc.tensor.matmul(aALL[:, 32 * j:32 * j + 32],
                                     p2Tb[:, 128 * j:128 * j + 128],
                                     whT2_sb[:], start=True, stop=True)
                nc.scalar.activation(eposAll[:], aALL[:], ExpF, scale=-1.0)
                posout = psA.tile([32, 66], F32, tag="sm")
                for j in range(8):
                    nc.tensor.matmul(posout[:], eposAll[:, 32 * j:32 * j + 32],
                                     va16[j // 2][:, j % 2, :],
                                     start=(j == 0), stop=(j == 7))
                recipZp = wk.tile([2, 1], F32)
                nc.vector.reciprocal(recipZp[:], posout[0:2, 64:65])
                gz2 = wk.tile([2, 1], F32)
                nc.vector.tensor_mul(gz2[:], recipZp[:], sg2m_sb[:])
                gpos2 = wk.tile([2, 64], F16)
                nc.vector.tensor_scalar_mul(gpos2[:], posout[0:2, 0:64],
                                            gz2[:, 0:1])
                gposTp = psA.tile([64, 2], F16, tag="sm")
                nc.tensor.transpose(gposTp[:], gpos2[:], id64r[0:2, 0:2])
                gposT2 = wk.tile([128, 1], F32)
                nc.vector.tensor_copy(gposT2[0:64, :], gposTp[:, 0:1])
                nc.vector.tensor_copy(gposT2[64:128, :], gposTp[:, 1:2])

                if stage < 3:
                    continue
                # ---- attention per head ----
                oT = wk.tile([128, 2, 512], F16)
                for i in range(2):
                    avh = psW.tile([128, 2, 512], F32, tag="av", bufs=1,
                                   name=f"avh{i}")
                    for p in range(4):
                        E8 = eP.tile([128, 2, T], F8, tag="E",
                                     name=f"E8_{i}_{p}")
                        for n in range(2):
                            sl = slice(512 * n, 512 * n + 512)
                            S2 = psW.tile([128, 2, 512], F32, tag="wide",
                                          name=f"S2_{i}_{p}_{n}")
                            for jj in range(2):
                                m = 2 * p + jj
                                nc.tensor.matmul(
                                    S2[:, jj, :],
                                    k8t[:, :, 128 * m:128 * m + 128],
                                    q8h[i][:, :, sl],
                                    start=True, stop=True, perf_mode=DR)
                            nc.scalar.activation(E8[:, :, sl], S2[:],
                                                 ExpF, scale=0.125)
                        for n in range(2):
                            sl = slice(512 * n, 512 * n + 512)
                            nc.tensor.matmul(avh[:, n, :], va8[p][:],
                                             E8[:, :, sl],
                                             start=(p == 0), stop=(p == 3),
                                             perf_mode=DR)
                    # avh rows 64:128 all hold 4Z (64 ones-columns in va8),
                    # so one reciprocal yields 1/(4Z) already broadcast
                    # across 64 partitions -- no cross-partition move.
                    zbS = wk.tile([64, 2, 512], F16, name=f"zbS{i}")
                    with nc.allow_low_precision(reason="1/Z feeds f16 mul"):
                        nc.vector.reciprocal(zbS[:], avh[64:128, :, :])
                    nc.vector.scalar_tensor_tensor(
                        oT[64 * i:64 * i + 64, :, :],
                        in0=avh[0:64, :, :],
                        scalar=c64_sb[:, i:i + 1],
                        in1=zbS[:],
                        op0=AluMult, op1=AluMult)
                    nc.vector.tensor_scalar_add(
                        oT[64 * i:64 * i + 64, :, :],
                        oT[64 * i:64 * i + 64, :, :],
                        gposT2[64 * i:64 * i + 64, 0:1])

                if stage < 4:
                    continue
                # ---- output projection (partial over this core's 128 o-cols)
                # Pool can't read PSUM, so the copies split ACT/DVE.
                outp_ap = outp_d.ap()
                for j in range(8):
                    lhs = oT[:, j // 4, 128 * (j % 4):128 * (j % 4) + 128]
                    outS = oP.tile([128, T], F16, tag="outS", name=f"oS{j}")
                    for n in range(2):
                        sl = slice(512 * n, 512 * n + 512)
                        po = psA.tile([128, 512], F32, tag="sm",
                                      name=f"po{j}_{n}")
                        nc.tensor.matmul(po[:], lhs, wo_sb[:, sl],
                                         start=True, stop=True)
                        if (2 * j + n) % 8 < 3:
                            nc.scalar.copy(outS[:, sl], po[:])
                        else:
                            nc.vector.tensor_copy(outS[:, sl], po[:])
                    nc.scalar.dma_start(
                        out=outp_ap[128 * j:128 * j + 128, :], in_=outS[:])

    nc.compile()
    return nc


def _host_inputs(inputs):
    """Per-core in_maps from the full inputs."""
    x = np.asarray(inputs["x"], np.float32)
    pos = np.asarray(inputs["pos"], np.float32)
    Wq = np.asarray(inputs["Wq"], np.float32)
    Wk = np.asarray(inputs["Wk"], np.float32)
    Wv = np.asarray(inputs["Wv"], np.float32)
    Wo = np.asarray(inputs["Wo"], np.float32)
    Wp1 = np.asarray(inputs["Wp1"], np.float32)
    bp1 = np.asarray(inputs["bp1"], np.float32)
    Wp2 = np.asarray(inputs["Wp2"], np.float32)
    Wh = np.asarray(inputs["Wh"], np.float32)
    gate = np.asarray(inputs["gate"], np.float32)

    xT = np.ascontiguousarray(x[0].T)
    x8f = xT.astype(E4NP)
    xr8f = (xT - x8f.astype(np.float32)).astype(E4NP)
    # p-major [128, 8*T]: partition p holds chunks k=0..8 contiguously
    x8 = np.ascontiguousarray(
        x8f.reshape(8, 128, T).transpose(1, 0, 2).reshape(128, 8 * T))
    xr8 = np.ascontiguousarray(
        xr8f.reshape(8, 128, T).transpose(1, 0, 2).reshape(128, 8 * T))
    WSC = 8.0  # weight prescale: keeps fp8 weights out of the subnormal
    # range; undone via the /8 RoPE tables and the /8 v-copy.

    posTa = np.ones((PD + 1, T), np.float16)
    posTa[0:PD, :] = pos[0].T.astype(np.float16)
    wp1Ta = np.zeros((PD + 1, PD), np.float16)
    wp1Ta[0:PD, :] = Wp1.T.astype(np.float16)
    wp1Ta[PD, :] = bp1.astype(np.float16)
    wp2T = np.ascontiguousarray(Wp2.T).astype(np.float16)

    # RoPE tables in transposed layout, tiled 4x along partitions
    j = np.arange(HD // 2, dtype=np.float32)
    theta = (BASE ** (-2.0 * j / HD)).astype(np.float32)
    freqs = np.arange(T, dtype=np.float32)[:, None] * theta  # [T, 32]
    cosT = np.ascontiguousarray(np.cos(freqs).T.astype(np.float32)) / WSC
    sinT = np.ascontiguousarray(np.sin(freqs).T.astype(np.float32)) / WSC
    tabc = np.concatenate([cosT] * 4, 0).astype(np.float16)
    tabs = np.concatenate([sinT, -sinT, sinT, -sinT], 0).astype(np.float16)

    ones8 = np.full((128, 128), 4.0, E4NP)
    ones16 = np.full((128, 4), 4.0, np.float16)

    sig = 1.0 / (1.0 + np.exp(-gate))

    in_maps = []
    for c in range(NC):
        g = c // 2
        def pmaj(wT):
            # [DIM, 128] -> [128, 8*128] p-major chunk layout
            return np.ascontiguousarray(
                wT.reshape(8, 128, 128).transpose(1, 0, 2).reshape(128, DIM))

        wq8 = pmaj(np.ascontiguousarray(
            Wq[128 * c:128 * c + 128, :].T * WSC).astype(E4NP))
        kvT = np.ascontiguousarray(
            np.concatenate([Wk[64 * g:64 * g + 64, :],
                            Wv[64 * g:64 * g + 64, :]], 0).T * WSC)
        wkvA0 = kvT.astype(E4NP)
        wkvB = pmaj((kvT - wkvA0.astype(np.float32)).astype(E5NP))
        wkvA = pmaj(wkvA0)
        wo_c = np.ascontiguousarray(Wo[:, 128 * c:128 * c + 128].T
                                    ).astype(np.float16)
        whT2_c = np.zeros((PF, 32), np.float16)
        whT2_c[:, 0:2] = Wh[2 * c:2 * c + 2, :].T
        c64 = np.zeros((64, 2), np.float32)
        c64[:, 0] = (1.0 - sig[2 * c]) * 4.0
        c64[:, 1] = (1.0 - sig[2 * c + 1]) * 4.0
        sg2m = (sig[2 * c:2 * c + 2] * 4.0).reshape(2, 1).copy()
        in_maps.append({
            "x8": x8, "xr8": xr8, "wq8": wq8, "wkvA": wkvA, "wkvB": wkvB,
            "wo": wo_c, "posTa": posTa, "wp1Ta": wp1Ta, "wp2T": wp2T,
            "whT2": whT2_c, "c64": c64, "sg2m": sg2m,
            "ones8": ones8, "ones16": ones16, "tabc": tabc, "tabs": tabs,
        })
    return in_maps


def get_program(reps=1, stage=99):
    key = f"nc{reps}_{stage}"
    if key not in _CACHE:
        _CACHE[key] = _build_program(reps, stage)
    return _CACHE[key]


def kernel(**inputs) -> np.ndarray:
    nc = get_program()
    in_maps = _host_inputs(inputs)
    res = bass_utils.run_bass_kernel_spmd(nc, in_maps, list(range(NC)))
    out = np.zeros((T, DIM), np.float32)
    for c in range(NC):
        out += res.results[c]["outp"].astype(np.float32)
    out += np.asarray(inputs["bo"], np.float32)
    return out.reshape(1, T, DIM)
